# revision 3
# baseline (speedup 1.0000x reference)
"""GNN message passing (copy_u + segment_sum) on 8 Trainium2 cores.

Strategy (edge/data parallel, per the sharding hint):
  - Host: sort edges by dst; core c owns dst range [c*N/8, (c+1)*N/8).
  - Host: pack src_emb rows as [bf16(hi) | bf16(src-hi)] pairs (256B/row, exact
    to ~1e-5 rel) and gather per-edge message rows per core ("src_emb ...
    gathered per partition"), laid out partition-major so device DMAs are
    contiguous 32KB runs per partition.
  - Device (per core): stream message tiles; for each bin of <=128 dst rows /
    S*128 edge slots, build a one-hot [edge x dst-slot] matrix on DVE
    (dstloc == iota), then segment-sum via PE matmuls accumulating hi+lo into
    one PSUM bank; copy PSUM->SBUF, batch-store bins per group.
  - Host: scatter-add the [B*128, 64] bin blocks back to the full output.
"""
import sys
sys.path.insert(0, "/opt/trn_rl_repo")
import numpy as np
import ml_dtypes

import concourse.bass as bass
import concourse.bacc as bacc
import concourse.mybir as mybir
import concourse.tile as tile
from concourse.bass_utils import run_bass_kernel_spmd

NCORES = 8
S = 9                # subtiles (of 128 edge slots) per bin
CAP = S * 128        # edge slots per bin
PAD_LOC = 200.0      # dstloc sentinel -> one-hot row all zeros
BF16 = ml_dtypes.bfloat16

_kernel_cache = {}


def _build_kernel(B):
    """Device program: uniform over cores; B bins of S subtiles each."""
    bf16 = mybir.dt.bfloat16
    f32 = mybir.dt.float32
    nc = bacc.Bacc("TRN2", target_bir_lowering=False, debug=False,
                   num_devices=NCORES)
    msg = nc.declare_dram_parameter("msg", [128, B * CAP], bf16, isOutput=False)
    dstloc = nc.declare_dram_parameter("dstloc", [128, B * S], bf16, isOutput=False)
    iota = nc.declare_dram_parameter("iota", [128, CAP], bf16, isOutput=False)
    outp = nc.declare_dram_parameter("outp", [128, B * 64], f32, isOutput=True)

    G = 14  # bins per DMA group
    n_groups = (B + G - 1) // G

    with tile.TileContext(nc) as tc:
        with tc.tile_pool(name="const", bufs=1) as cpool, \
             tc.tile_pool(name="msgs", bufs=3) as mpool, \
             tc.tile_pool(name="oh", bufs=4) as ohpool, \
             tc.tile_pool(name="acc", bufs=8, space="PSUM") as ppool, \
             tc.tile_pool(name="ost", bufs=3) as opool:
            iota_t = cpool.tile([128, CAP], bf16)
            nc.sync.dma_start(out=iota_t[:], in_=iota[:])
            dstloc_t = cpool.tile([128, B * S], bf16)
            nc.sync.dma_start(out=dstloc_t[:], in_=dstloc[:])
            iota3d = iota_t[:].rearrange("p (s f) -> p s f", s=S)

            for g in range(n_groups):
                g0 = g * G
                gs = min(G, B - g0)
                mt = mpool.tile([128, gs * CAP], bf16, tag="mt")
                nc.sync.dma_start(out=mt[:], in_=msg[:, g0 * CAP:(g0 + gs) * CAP])
                ot = opool.tile([128, gs * 64], f32, tag="ot")
                for lb in range(gs):
                    b = g0 + lb
                    oh = ohpool.tile([128, S, 128], bf16)
                    nc.vector.tensor_tensor(
                        out=oh[:],
                        in0=dstloc_t[:, b * S:(b + 1) * S].to_broadcast([128, S, 128]),
                        in1=iota3d,
                        op=mybir.AluOpType.is_equal,
                    )
                    ps = ppool.tile([128, 64], f32)
                    for s in range(S):
                        base = lb * CAP + s * 128
                        nc.tensor.matmul(ps[:], oh[:, s, :], mt[:, base:base + 64],
                                         start=(s == 0), stop=False)
                        nc.tensor.matmul(ps[:], oh[:, s, :], mt[:, base + 64:base + 128],
                                         start=False, stop=(s == S - 1))
                    nc.vector.tensor_copy(out=ot[:, lb * 64:(lb + 1) * 64], in_=ps[:])
                nc.sync.dma_start(out=outp[:, g0 * 64:(g0 + gs) * 64], in_=ot[:])
    nc.compile()
    return nc


def _pack_core(d_local, s_local, n_dst_local):
    """Greedy bins: <=128 distinct dst rows and <=CAP edges per bin.
    Returns (srcs [B,CAP] int64, locs [B,CAP] uint8->float, rows [B,128] int64
    with n_dst_local as trash)."""
    n = len(d_local)
    bins = []
    if n:
        firsts = np.flatnonzero(np.concatenate(([True], d_local[1:] != d_local[:-1])))
        nf = len(firsts)
        start = 0
        while start < n:
            j0 = np.searchsorted(firsts, start, side="right") - 1
            lim = firsts[j0 + 128] if j0 + 128 < nf else n
            end = min(start + CAP, lim)
            bins.append((start, end))
            start = end
    B = len(bins)
    srcs = np.zeros((B, CAP), dtype=np.int64)
    locs = np.full((B, CAP), PAD_LOC, dtype=np.float32)
    rows = np.full((B, 128), n_dst_local, dtype=np.int64)
    for i, (st, en) in enumerate(bins):
        m = en - st
        u, inv = np.unique(d_local[st:en], return_inverse=True)
        srcs[i, :m] = s_local[st:en]
        locs[i, :m] = inv.astype(np.float32)
        rows[i, :len(u)] = u
    return srcs, locs, rows


def kernel(src_emb, edge_src, edge_dst, num_dst):
    src_emb = np.asarray(src_emb, dtype=np.float32)
    edge_src = np.asarray(edge_src).astype(np.int64)
    edge_dst = np.asarray(edge_dst).astype(np.int64)
    n_dst = int(num_dst)
    n_src, d = src_emb.shape
    assert d == 64

    # hi/lo bf16 split: hi + lo == src exactly to ~2^-17 relative
    hi = src_emb.astype(BF16)
    lo = (src_emb - hi.astype(np.float32)).astype(BF16)
    packed = np.concatenate([hi, lo], axis=1)  # [n_src, 128] bf16

    # dst-sorted edge partition across cores
    order = np.argsort(edge_dst, kind="stable")
    ds = edge_dst[order]
    ss = edge_src[order]
    per = (n_dst + NCORES - 1) // NCORES
    cuts = np.searchsorted(ds, np.arange(1, NCORES) * per)
    d_parts = np.split(ds, cuts)
    s_parts = np.split(ss, cuts)

    cores = []
    for c in range(NCORES):
        dl = d_parts[c] - c * per
        nl = min(per, n_dst - c * per)
        cores.append(_pack_core(dl, s_parts[c], nl))
    B = max(cr[0].shape[0] for cr in cores)

    iota_np = np.tile(np.arange(128, dtype=np.float32), S)[None, :].repeat(128, 0).astype(BF16)

    in_maps = []
    rows_g = []
    for c, (srcs, locs, rows) in enumerate(cores):
        b0 = srcs.shape[0]
        if b0 < B:
            srcs = np.concatenate([srcs, np.zeros((B - b0, CAP), np.int64)])
            locs = np.concatenate([locs, np.full((B - b0, CAP), PAD_LOC, np.float32)])
            nl = min(per, n_dst - c * per)
            rows = np.concatenate([rows, np.full((B - b0, 128), nl, np.int64)])
        # [128, B*S*128] partition-major messages
        msg_np = packed[srcs.reshape(B * S, 128).T].reshape(128, -1)
        dstloc_np = locs.reshape(B * S, 128).T.astype(BF16).copy()
        in_maps.append({"msg": msg_np, "dstloc": dstloc_np, "iota": iota_np})
        nl = min(per, n_dst - c * per)
        # local trash sentinel nl -> dedicated global trash slot n_dst + c
        rows_g.append(np.where(rows == nl, n_dst + c, rows + c * per))

    if B not in _kernel_cache:
        _kernel_cache[B] = _build_kernel(B)
    nc = _kernel_cache[B]
    res = run_bass_kernel_spmd(nc, in_maps, core_ids=list(range(NCORES)))

    full = np.zeros((n_dst + NCORES, 64), dtype=np.float32)
    for c in range(NCORES):
        blocks = res.results[c]["outp"].reshape(128, B, 64).transpose(1, 0, 2)
        np.add.at(full, rows_g[c].ravel(), blocks.reshape(B * 128, 64))
    return full[:n_dst]


if __name__ == "__main__":
    rng = np.random.default_rng(1)
    ns, nd, e = 1000, 1000, 5000
    semb = rng.standard_normal((ns, 64), dtype=np.float32)
    es = rng.integers(0, ns, e)
    ed = rng.integers(0, nd, e)
    got = kernel(src_emb=semb, edge_src=es, edge_dst=ed, num_dst=nd)
    exp = np.zeros((nd, 64), np.float32)
    np.add.at(exp, ed, semb[es])
    rel = np.abs(got - exp).max() / np.abs(exp).max()
    print("small-case rel err:", rel)
